# revision 33
# baseline (speedup 1.0000x reference)
"""Trainium2 Bass kernel for DiagonalGMMPosterior (vq_codebook).

Reference computation (per batch b, descriptor n, cluster k):
    dist[k,n]  = sum_d (x[d,n] - mu_n[k,d])^2 * exp(-log_sigma[k,d])
    logits     = -dist + log_alpha[k] - 0.5 * sum_d log_sigma[k,d]
    out[k,n]   = softmax_k(logits)

Device strategy (8 NeuronCores, data-parallel over the batch axis):
  * Host folds all (K,D) parameter math into two GEMM weight matrices and a
    per-cluster constant, then CENTERS them across K.  Softmax is invariant
    to per-n shifts, so subtracting the K-mean of the logits (a rank-1
    update folded into the weights on host) bounds the logits and removes
    the need for a per-n max reduction entirely.
  * x is shipped to the device as fp16 (halves HBM load traffic).  The
    k-dependent part of the fp16 rounding error is ~1e-3 in logit space
    (the k-independent part cancels in softmax), far inside the 2e-2 gate.
  * K=64 but SBUF/PSUM have 128 partitions, so consecutive 512-column
    blocks are STACKED: even blocks' logits land on PSUM partitions 0-63
    (PE column tile 0), odd blocks' on 64-127 (PE column tile 64).  Every
    post-GEMM op (exp / ones-matmul / reciprocal / multiply) then runs on
    128 partitions at half the free size.
  * Work is grouped two 1024-column tiles at a time over a two-bank
    [128,1024] PSUM tile, with the 10 matmuls of a group ordered so each
    of the 5 weight tiles (w1@col0, w1@col64, w2@col0, w2@col64, ones) is
    loaded once and the second matmul reuses it via ldweights=False.
  * Engine balance (measured ns/elem-col: Act 0.83, DVE 0.85, GpSimd 2.8):
    square + reciprocal on VectorE, exp on ScalarE (bf16 out), the final
    multiply on GpSimd (VectorE is the pace-setter; the drain-phase
    multiplies go on VectorE instead, which is idle by then).  The
    denominator is a block-diag ones-matmul on TensorE (sums each
    64-partition block AND broadcasts in one pass).
  * The device writes the stacked layout verbatim ([128, N/2] per batch);
    the host de-interleaves with one numpy transpose.  Input DMAs issue on
    the Sync queue, output DMAs on the Scalar queue.
"""

import numpy as np

import concourse.bacc as bacc
import concourse.bass as bass
import concourse.tile as tile
from concourse import mybir
from concourse.bass_utils import run_bass_kernel_spmd

B, D, N, K = 16, 128, 16384, 64
NCORES = 8
BPC = B // NCORES   # batches per core
NT = 512            # one PSUM bank of fp32; stacked block width
GROUP = 4 * NT      # x columns per group (two stacked 1024-col tiles)
NG = N // GROUP     # groups per batch row

F32 = mybir.dt.float32
F16 = mybir.dt.float16
BF16 = mybir.dt.bfloat16

_CACHE = {}


def _build_nc():
    # Bacc (not raw Bass): its compile() pass legalizes Tile's multi-wait
    # instructions down to the 1-wait-per-instruction hardware limit.
    nc = bacc.Bacc("TRN2", target_bir_lowering=False, debug=False)
    x_in = nc.declare_dram_parameter("x", [BPC, D, N], F16, isOutput=False)
    w1_in = nc.declare_dram_parameter("w1", [D, K], F16, isOutput=False)
    w2_in = nc.declare_dram_parameter("w2", [D, K], F16, isOutput=False)
    cc_in = nc.declare_dram_parameter("cc", [2 * K, 1], F32, isOutput=False)
    ones_in = nc.declare_dram_parameter("ones_bd", [2 * K, 2 * K], BF16,
                                        isOutput=False)
    # stacked device layout: partition 64h+k, column 1024g+512p+c holds
    # posterior[k, 2048g+1024p+512h+c]; the host de-interleaves.  fp16
    # halves the store traffic (~5e-4 rounding on values in [0,1]; uint8
    # fixed-point would halve it again but walrus codegen rejects uint8
    # elementwise outputs regardless of the producing op).
    out_ext = nc.declare_dram_parameter("out", [BPC, 2 * K, N // 2], F16,
                                        isOutput=True)

    with tile.TileContext(nc) as tc:
        with (
            tc.tile_pool(name="consts", bufs=1) as consts,
            tc.tile_pool(name="xp", bufs=5) as xp,
            tc.tile_pool(name="qp", bufs=5) as qp,
            tc.tile_pool(name="ep", bufs=6) as ep,
            tc.tile_pool(name="op", bufs=6) as op,
            tc.tile_pool(name="rp", bufs=6) as rp,
            tc.tile_pool(name="pd", bufs=2, space="PSUM") as pdp,
            tc.tile_pool(name="pb", bufs=2, space="PSUM") as pbp,
        ):
            groups = [(b, g) for b in range(BPC) for g in range(NG)]
            NPG = len(groups)
            st = [dict() for _ in range(NPG)]

            def s0_load(i):
                b, g = groups[i]
                n0 = g * GROUP
                # twin [128,1024] tiles: full-tile squares engage the DVE 2x
                # mode (sliced or 2048-wide ops measurably do not)
                xt0 = xp.tile([D, GROUP // 2], F16, tag="xt0")
                nc.sync.dma_start(
                    out=xt0, in_=x_in[b, :, n0 : n0 + GROUP // 2]
                )
                xt1 = xp.tile([D, GROUP // 2], F16, tag="xt1")
                nc.sync.dma_start(
                    out=xt1, in_=x_in[b, :, n0 + GROUP // 2 : n0 + GROUP]
                )
                st[i]["xt"] = (xt0, xt1)

            # the first x tiles go ahead of the parameter loads: squares and
            # the GEMMs' moving data are on the critical path at fill time,
            # the weights are not needed until several microseconds in
            PRE = 2
            for i in range(PRE):
                s0_load(i)

            w1_sb = consts.tile([D, K], F16)
            nc.sync.dma_start(out=w1_sb, in_=w1_in[:, :])
            w2_sb = consts.tile([D, K], F16)
            nc.sync.dma_start(out=w2_sb, in_=w2_in[:, :])
            cc_sb = consts.tile([2 * K, 1], F32)
            nc.sync.dma_start(out=cc_sb, in_=cc_in[:, :])
            ones_bd = consts.tile([2 * K, 2 * K], BF16)
            nc.sync.dma_start(out=ones_bd, in_=ones_in[:, :])

            def s1_square(i):
                xt0, xt1 = st[i]["xt"]
                # one square on ScalarE, one on VectorE — whole-tile ops on
                # separate engines.  (Shifting more squares onto VectorE
                # lowers its nominal busy time but lengthens the critical
                # path feeding the GEMMs and measures WORSE end to end.)
                xsq0 = qp.tile([D, GROUP // 2], F16, tag="xsq0")
                nc.scalar.activation(
                    out=xsq0, in_=xt0,
                    func=mybir.ActivationFunctionType.Square,
                )
                xsq1 = qp.tile([D, GROUP // 2], F16, tag="xsq1")
                nc.vector.tensor_mul(xsq1, xt1, xt1)
                st[i]["xsq"] = (xsq0, xsq1)

            def s2_dist(i):
                xt, xsq = st[i]["xt"], st[i]["xsq"]
                # two stacked 1024-col tiles over one two-bank PSUM tile:
                # x columns 1024p + 512h + c -> PSUM partition block h,
                # column 512p + c.  Matmuls are grouped per weight tile so
                # the second matmul reuses the loaded weights.
                pd_t = pdp.tile([2 * K, 2 * NT], F32, tag="pd")
                for w_sb, mv, start in ((w1_sb, xsq, True), (w2_sb, xt, False)):
                    for h in range(2):
                        pr = slice(h * K, (h + 1) * K)
                        for p in range(2):
                            j = 2 * p + h
                            src = mv[j // 2][:, (j % 2) * NT : (j % 2 + 1) * NT]
                            mm = nc.tensor.matmul(
                                pd_t[pr, p * NT : (p + 1) * NT],
                                w_sb[:, :],
                                src,
                                start=start, stop=not start,
                            )
                            if p == 1:
                                mm.ldweights = False
                st[i]["pd"] = pd_t

            def s3_exp(i):
                pd_t = st[i].pop("pd")
                et = ep.tile([2 * K, 2 * NT], BF16, tag="et")
                nc.scalar.activation(
                    out=et, in_=pd_t,
                    func=mybir.ActivationFunctionType.Exp,
                    bias=cc_sb, scale=1.0,
                )
                st[i]["et"] = et
                st[i].pop("xt")
                st[i].pop("xsq")

            def s4_den(i):
                et = st[i]["et"]
                # denominator: block-diag ones sums each 64-partition block
                # separately AND broadcasts the sum to all 64 partitions of
                # that block; two 512-col streams share one weight load
                pb_t = pbp.tile([2 * K, 2 * NT], F32, tag="pb")
                for p in range(2):
                    mm = nc.tensor.matmul(
                        pb_t[:, p * NT : (p + 1) * NT],
                        ones_bd[:, :],
                        et[:, p * NT : (p + 1) * NT],
                        start=True, stop=True,
                    )
                    if p == 1:
                        mm.ldweights = False
                st[i]["pb"] = pb_t

            def s5_recip(i):
                pb_t = st[i].pop("pb")
                # ~18-bit-accurate custom-DVE reciprocal; the sum is always
                # >= 1 (mean-centered logits), so the undefined edge cases
                # (0/denorm/inf) cannot occur.  Emitted via _custom_dve to
                # write bf16 directly (the wrapper's fp32-out assert guards
                # the seed's INPUT bit layout; the output write-path cast is
                # fine) — 2-byte r makes every multiply all-2-byte.
                from concourse.dve_ops import (
                    RECIP_APPROX_FAST_CONSTS,
                    RECIPROCAL_APPROX_FAST,
                )
                r_all = rp.tile([2 * K, 2 * NT], BF16, tag="r")
                c = RECIP_APPROX_FAST_CONSTS
                nc.vector._custom_dve(
                    RECIPROCAL_APPROX_FAST,
                    out=r_all, in0=pb_t,
                    s0=c["s0"], s1=c["s1"], imm2=c["imm2"],
                )
                st[i]["r"] = r_all

            def s6_mult(i):
                et, r_all = st[i].pop("et"), st[i].pop("r")
                ot = op.tile([2 * K, 2 * NT], F16, tag="ot")
                # GpSimd absorbs the multiply in steady state; the last few
                # groups go on VectorE, which is idle during the drain
                if i >= NPG - 3:
                    nc.vector.tensor_mul(ot, et, r_all)
                else:
                    nc.gpsimd.tensor_mul(ot, et, r_all)
                st[i]["ot"] = ot

            def s7_store(i):
                b, g = groups[i]
                c0 = g * 2 * NT
                ot = st[i].pop("ot")
                nc.sync.dma_start(
                    out=out_ext[b, :, c0 : c0 + 2 * NT], in_=ot[:, :]
                )

            stages = [
                s0_load, s1_square, s2_dist, s3_exp,
                s4_den, s5_recip, s6_mult, s7_store,
            ]
            NS = len(stages)
            # downstream stages emitted first within each tick so no
            # engine's in-order queue blocks a later group's earlier stage
            for tick in range(NPG + NS - 1):
                for k in reversed(range(NS)):
                    i = tick - k
                    if 0 <= i < NPG and not (k == 0 and i < PRE):
                        stages[k](i)
    nc.compile()
    return nc


def _host_params(mu, log_sigma, log_alpha):
    mu64 = mu.astype(np.float64)
    mu_n = mu64 / np.maximum(
        np.linalg.norm(mu64, axis=1, keepdims=True), 1e-12
    )
    sinv = np.exp(-log_sigma.astype(np.float64))  # (K, D)
    a1 = -sinv                                    # coeff of x^2 in logits
    a2 = 2.0 * mu_n * sinv                        # coeff of x
    c = (
        -np.sum(mu_n * mu_n * sinv, axis=1)
        + log_alpha.astype(np.float64)
        - 0.5 * np.sum(log_sigma.astype(np.float64), axis=1)
    )
    # center across K: softmax is invariant to per-n shifts, and this keeps
    # the on-device logits within exp()'s comfortable range
    a1c = a1 - a1.mean(axis=0, keepdims=True)
    a2c = a2 - a2.mean(axis=0, keepdims=True)
    ccv = c - c.mean()
    w1 = np.ascontiguousarray(a1c.T, dtype=np.float16)  # (D, K)
    w2 = np.ascontiguousarray(a2c.T, dtype=np.float16)  # (D, K)
    cc = np.tile(ccv.astype(np.float32).reshape(K, 1), (2, 1))  # (2K, 1)
    return w1, w2, cc


def _in_maps(x, mu, log_sigma, log_alpha):
    x = np.asarray(x).astype(np.float16)
    w1, w2, cc = _host_params(
        np.asarray(mu), np.asarray(log_sigma), np.asarray(log_alpha)
    )
    from ml_dtypes import bfloat16
    ones_bd = np.kron(
        np.eye(2, dtype=np.float32), np.ones((K, K), dtype=np.float32)
    ).astype(bfloat16)
    return [
        {
            "x": np.ascontiguousarray(x[i * BPC : (i + 1) * BPC]),
            "w1": w1,
            "w2": w2,
            "cc": cc,
            "ones_bd": ones_bd,
        }
        for i in range(NCORES)
    ]


def kernel(x, mu, log_sigma, log_alpha):
    if "nc" not in _CACHE:
        _CACHE["nc"] = _build_nc()
    nc = _CACHE["nc"]
    in_maps = _in_maps(x, mu, log_sigma, log_alpha)
    res = run_bass_kernel_spmd(nc, in_maps, list(range(NCORES))).results
    outs = []
    for i in range(NCORES):
        dev = np.asarray(res[i]["out"])  # [BPC, 2K, N//2] fp16
        # partition 64h+k, column 1024g+512p+c  ->  [k, 2048g+1024p+512h+c]
        v = dev.reshape(BPC, 2, K, NG, 2, NT)
        outs.append(
            np.transpose(v, (0, 2, 3, 4, 1, 5)).reshape(BPC, K, N)
        )
    return np.concatenate(outs, axis=0).astype(np.float32)
